# revision 16
# baseline (speedup 1.0000x reference)
"""TP(heads)xDP(batch) sharded causal GQA attention block for 8 trn2 cores.

Each core c handles batch b=c//4 and head group g=c%4 (8 q heads, 2 kv heads).
Per-core pipeline (fused over 4 query chunks of 512):
  qkv = Wqkv_c @ x_b^T  (bf16 matmuls, outputs [feature, token] layout)
  RoPE on q/k (DMA partition swap + bf16 DVE mul/mul/add)
  scores_T[kj, qi] = k^T q  (bf16, 2 heads packed in the 128-row PE array)
  exp on ScalarE (no max subtraction; scores are ~N(0,1) after 1/8 scaling)
  out_aug[65, qi] = [v; ones]^T exp  (bf16; row 64 = softmax denominator)
  normalize via one batched DVE reciprocal + gpsimd partition_broadcast
  partial_T[dout, t] = Wo_c^T attn  (bf16), host sums the 8 partials.

The ScalarE exp (~1 us per kj tile) is slower than the PE's attention work
per kj tile (~0.65 us), so the PE idles waiting on exp unless it has filler:
o_proj matmuls of chunk tci-1 and the qkv projection of chunk tci+1 are
emitted interleaved into the attention kj loop to keep the PE dense (dense
PE = no HAM clock-gate re-throttle, which otherwise costs ~40% on matmuls).
"""
import sys
sys.path.insert(0, "/opt/trn_rl_repo")
from contextlib import ExitStack

import numpy as np
import ml_dtypes

B, L, D = 2, 2048, 2048
NH, NKV, HD = 32, 8, 64
ROPE_BASE = 10000.0
SCALE = HD ** -0.5
TC, TCW = 4, 512      # query/token chunks
NKD = 16              # d contraction tiles
NOT = 6               # output tiles per core (4 q packs, k pack, v pack)
NKJ = 16              # key tiles

BF16 = ml_dtypes.bfloat16

_cached = {}


def _build_nc():
    import concourse.bacc as bacc
    import concourse.tile as tile
    import concourse.mybir as mybir
    from concourse import library_config

    F32 = mybir.dt.float32
    BF = mybir.dt.bfloat16
    AF = mybir.ActivationFunctionType

    nc = bacc.Bacc("TRN2", debug=False)
    xh_ap = nc.dram_tensor("xh", (TC, 128, NKD * TCW), BF, kind="ExternalInput").ap()
    wq_ap = nc.dram_tensor("wq", (128, NKD * NOT * 128), BF, kind="ExternalInput").ap()
    wo_ap = nc.dram_tensor("wo", (128, 4 * NKD * 128), BF, kind="ExternalInput").ap()
    ccss_ap = nc.dram_tensor("ccss", (128, 2 * L), BF, kind="ExternalInput").ap()
    msk_ap = nc.dram_tensor("msk", (128, 8 * TCW), BF, kind="ExternalInput").ap()
    id2_ap = nc.dram_tensor("id2", (128, 64), BF, kind="ExternalInput").ap()
    out_ap = nc.dram_tensor("outp", (TC, NKD, 128, TCW), BF, kind="ExternalOutput").ap()

    with tile.TileContext(nc) as tcx, ExitStack() as ctx:
        pc = ctx.enter_context(tcx.tile_pool(name="const", bufs=1))
        px = ctx.enter_context(tcx.tile_pool(name="x", bufs=2))
        pw = ctx.enter_context(tcx.tile_pool(name="work", bufs=1))
        psc = ctx.enter_context(tcx.tile_pool(name="psc", bufs=2, space="PSUM"))
        paug = ctx.enter_context(tcx.tile_pool(name="paug", bufs=1, space="PSUM"))
        pmm = ctx.enter_context(tcx.tile_pool(name="pmm", bufs=2, space="PSUM"))

        # wq is laid out ot-major on the host: one contiguous 2KB/partition
        # block per ot, DMA'd in consumption order so the first qkv matmul
        # group only waits for its own block (initial loads are HBM-BW bound).
        wq_t = pc.tile([128, NKD * NOT * 128], BF)
        wo_t = pc.tile([128, 4 * NKD * 128], BF)
        ccss_t = pc.tile([128, 2 * L], BF)
        msk_t = pc.tile([128, 8 * TCW], BF)
        id2_t = pc.tile([128, 64], BF)

        def load_wq(ot):
            w = NKD * 128
            cw = w // 4
            for c in range(4):
                lo = ot * w + c * cw
                nc.sync.dma_start(wq_t[:, lo:lo + cw], wq_ap[:, lo:lo + cw])

        def load_consts():
            nc.sync.dma_start(id2_t[:, :], id2_ap[:, :])
            for i in range(4):
                nc.sync.dma_start(ccss_t[:, i * L // 2:(i + 1) * L // 2],
                                  ccss_ap[:, i * L // 2:(i + 1) * L // 2])
            for i in range(2):
                nc.sync.dma_start(msk_t[:, i * 4 * TCW:(i + 1) * 4 * TCW],
                                  msk_ap[:, i * 4 * TCW:(i + 1) * 4 * TCW])

        def load_wo():
            wow = 4 * NKD * 128 // 8
            for i in range(8):
                nc.sync.dma_start(wo_t[:, i * wow:(i + 1) * wow], wo_ap[:, i * wow:(i + 1) * wow])

        kpack = pc.tile([128, L], BF)            # k (2 kv heads stacked), RoPE'd
        vaug = pc.tile([128, 2 * NKJ * 65], BF)  # [v | ones] per (kv, kj)
        nc.vector.memset(vaug[:, 64::65], 1.0)   # ones columns
        ones1 = pc.tile([1, 64], BF)             # K=1 broadcast-matmul weights
        nc.vector.memset(ones1[:, :], 1.0)

        _pairs = [None] * 4
        xt_tiles = {}
        qraw_tiles = {}

        def load_xt(t):
            xtt = px.tile([128, NKD * TCW], BF, tag="xt")
            xw = NKD * TCW // 8
            for i in range(8):
                nc.sync.dma_start(xtt[:, i * xw:(i + 1) * xw], xh_ap[t][:, i * xw:(i + 1) * xw])
            xt_tiles[t] = xtt

        def rope_batched(raw, dest_ap, t, nrep):
            """dest = raw*CC + swap32(raw)*SS; raw is [128, nrep*TCW] bf16.
            Partition 32-block swap done via sbuf->sbuf DMA."""
            w = nrep * TCW
            swp = pw.tile([128, 4 * TCW], BF, tag="swp", bufs=2)
            for blk in range(4):
                nc.sync.dma_start(swp[32 * blk:32 * blk + 32, 0:w],
                                  raw[32 * (blk ^ 1):32 * (blk ^ 1) + 32, 0:w])
            cs = ccss_t[:, t * TCW:(t + 1) * TCW]
            ss = ccss_t[:, L + t * TCW:L + (t + 1) * TCW]
            for rp in range(nrep):
                rs = slice(rp * TCW, (rp + 1) * TCW)
                nc.vector.tensor_mul(raw[:, rs], raw[:, rs], cs)
                nc.vector.tensor_mul(swp[:, rs], swp[:, rs], ss)
                nc.vector.tensor_add(dest_ap[:, rs] if nrep > 1 else dest_ap,
                                     raw[:, rs], swp[:, rs])

        def emit_qkv_units(t):
            """Thunks emitting the qkv projection of chunk t in ~4-MM chunks,
            plus per-ot epilogues (psum copy, k-rope, v transposes, q-rope)."""
            units = []
            ps_holder = {}

            def mm_chunk(ot, dtc):
                def f():
                    if dtc == 0:
                        ps_holder[ot] = pmm.tile([128, TCW], F32, tag="mm", name=f"ps{ot}")
                    ps = ps_holder[ot]
                    for dt in range(4 * dtc, 4 * dtc + 4):
                        nc.tensor.matmul(
                            ps[:, :], wq_t[:, (ot * NKD + dt) * 128:(ot * NKD + dt + 1) * 128],
                            xt_tiles[t][:, dt * TCW:(dt + 1) * TCW],
                            start=(dt == 0), stop=(dt == NKD - 1))
                return f

            def epi(ot):
                def f():
                    ps = ps_holder.pop(ot)
                    if ot == 4:
                        kraw = pw.tile([128, TCW], BF, tag="kraw", bufs=2)
                        with tcx.high_priority():
                            nc.vector.tensor_copy(kraw[:, :], ps[:, :])
                        rope_batched(kraw, kpack[:, t * TCW:(t + 1) * TCW], t, 1)
                    elif ot == 5:
                        vch = pw.tile([128, TCW], BF, tag="vch", bufs=2)
                        ps_holder["v"] = vch
                        with tcx.high_priority():
                            nc.vector.tensor_copy(vch[:, :], ps[:, :])
                    else:
                        if "q" not in ps_holder:
                            ps_holder["q"] = pw.tile([128, 4 * TCW], BF, tag="qraw", bufs=2, name="qraw")
                            qraw_tiles[t] = ps_holder["q"]
                        with tcx.high_priority():
                            nc.vector.tensor_copy(
                                ps_holder["q"][:, ot * TCW:(ot + 1) * TCW], ps[:, :])
                return f

            def vtrans(j, jj):
                def f():
                    vch = ps_holder["v"]
                    kj = 4 * t + jj
                    tp = pmm.tile([128, 64], BF, tag="mm")
                    nc.tensor.transpose(
                        tp[:, :], vch[64 * j:64 * j + 64, jj * 128:(jj + 1) * 128],
                        id2_t[64 * j:64 * j + 64, :])
                    col = (j * NKJ + kj) * 65
                    with tcx.high_priority():
                        nc.vector.tensor_copy(vaug[:, col:col + 64], tp[:, :])
                return f

            def qrope(rp):
                def f():
                    qr = ps_holder["q"]
                    rs = slice(rp * TCW, (rp + 1) * TCW)
                    swp = pw.tile([128, TCW], BF, tag="qswp", bufs=3, name="swp")
                    for blk in range(4):
                        nc.sync.dma_start(swp[32 * blk:32 * blk + 32, :],
                                          qr[32 * (blk ^ 1):32 * (blk ^ 1) + 32, rs])
                    cs = ccss_t[:, t * TCW:(t + 1) * TCW]
                    ss = ccss_t[:, L + t * TCW:L + (t + 1) * TCW]
                    nc.vector.tensor_mul(qr[:, rs], qr[:, rs], cs)
                    nc.vector.tensor_mul(swp[:, :], swp[:, :], ss)
                    nc.vector.tensor_add(qr[:, rs], qr[:, rs], swp[:, :])
                return f

            for ot in (4, 5, 0, 1, 2, 3):
                for dtc in range(4):
                    units.append(mm_chunk(ot, dtc))
                units.append(epi(ot))
                if ot == 5:
                    for j in range(2):
                        for jj in range(4):
                            units.append(vtrans(j, jj))
            for rp in range(4):
                units.append(qrope(rp))
            return units

        def oproj_units(otc, pairs_):
            units = []

            def one(dt):
                def f():
                    po = pmm.tile([128, TCW], F32, tag="mm")
                    for kt in range(4):
                        nc.tensor.matmul(
                            po[:, :], wo_t[:, (kt * NKD + dt) * 128:(kt * NKD + dt + 1) * 128],
                            pairs_[kt][:, :],
                            start=(kt == 0), stop=(kt == 3))
                    ev = pw.tile([128, TCW], BF, tag="ev", bufs=6)
                    nc.vector.tensor_copy(ev[:, :], po[:, :])
                    nc.sync.dma_start(out_ap[otc, dt], ev[:, :])
                return f

            for dt in range(NKD):
                units.append(one(dt))
            return units

        def make_norm_units(augs, den8b):
            """Pre-create the pair tiles; return (pairs, units). Units emit the
            batched reciprocal + per-pack broadcast/normalize. They are run as
            fillers inside the NEXT chunk's attention loop so the in-order DVE /
            gpsimd queues interleave them with critical-path work."""
            pairs = [pw.tile([128, TCW], BF, tag="pair", bufs=12, name=f"pair{p}")
                     for p in range(4)]
            rcs = [pw.tile([1, TCW], BF, tag="rcs", bufs=9, name=f"rcs{i}")
                   for i in range(8)]
            units = []

            def recip_unit():
                with nc.allow_low_precision(reason="softmax recip in bf16"):
                    den8 = pw.tile([8, TCW], F32, tag="den8", bufs=2)
                    nc.vector.tensor_copy(den8[:, :], den8b[:, :])
                    rc8f = pw.tile([8, TCW], F32, tag="rc8f", bufs=2)
                    nc.vector.reciprocal_approx_fast(rc8f[:, :], den8[:, :])
                    rc8 = pw.tile([8, TCW], BF, tag="rc8", bufs=2)
                    nc.vector.tensor_copy(rc8[:, :], rc8f[:, :])
                    for i in range(8):
                        nc.sync.dma_start(rcs[i][:, :], rc8[i:i + 1, :])
            units.append(recip_unit)

            def pack_unit(p):
                def f():
                    with nc.allow_low_precision(reason="softmax normalize in bf16"):
                        pair = pairs[p]
                        augSA, augSB = augs[p]
                        bA = pmm.tile([64, TCW], F32, tag="mm", name="bA")
                        nc.tensor.matmul(bA[:, :], ones1[:, :], rcs[2 * p][:, :],
                                         start=True, stop=True)
                        nc.vector.tensor_mul(pair[0:64, :], augSA[0:64, :], bA[:, :])
                        bB = pmm.tile([64, TCW], F32, tag="mm", name="bB")
                        nc.tensor.matmul(bB[:, :], ones1[:, :], rcs[2 * p + 1][:, :],
                                         start=True, stop=True)
                        ob = pw.tile([64, TCW], BF, tag="ob", bufs=3)
                        nc.vector.tensor_mul(ob[:, :], augSB[0:64, :], bB[:, :])
                        nc.sync.dma_start(pair[64:128, :], ob[:, :])
                return f
            for p in range(4):
                units.append(pack_unit(p))
            return pairs, units

        norm_units_state = {"pending": []}
        deferred_oproj = []

        def _mk_norm(augs, den8b):
            pairs, units = make_norm_units(augs, den8b)
            norm_units_state["pending"] = units
            return pairs
        norm_units_state["mk"] = _mk_norm

        prev_pairs = None
        for tci in range(TC):
            if tci == 0:
                load_xt(0)
                for ot in (4, 5, 0, 1, 2, 3):
                    load_wq(ot)
                load_consts()
                for u in emit_qkv_units(0):
                    u()
                load_wo()
            if tci + 1 < TC:
                load_xt(tci + 1)

            early = []
            if tci + 1 < TC:
                early += emit_qkv_units(tci + 1)
            # norm units MUST be emitted before the o_proj units that read the
            # pair tiles they write (Tile deps come from emission order).
            late = list(norm_units_state["pending"])
            norm_units_state["pending"] = []
            if tci >= 1:
                u = oproj_units(tci - 1, prev_pairs)
                if tci == 2:
                    # shift half of oproj(1) into loop 3, which has no qkv
                    # filler (the loop with the most attention iterations)
                    late += u[:8]
                    deferred_oproj.append(u[8:])
                else:
                    late += u
            if tci == 3 and deferred_oproj:
                late += deferred_oproj.pop(0)

            qall = qraw_tiles.pop(tci)
            total_iters = 4 * (4 * tci + 4)
            it = 0
            consumed_e = 0
            consumed_l = 0

            # ---- attention for query chunk tci, all 4 packs ----
            augs = [None] * 4
            last_kj = 4 * tci + 3
            for p in range(4):
                qs = slice(p * TCW, (p + 1) * TCW)
                augA = paug.tile([65, TCW], F32, tag="augA")
                augB = paug.tile([65, TCW], F32, tag="augB")
                for kj in range(4 * tci + 4):
                    ks = slice(kj * 128, (kj + 1) * 128)
                    scp = psc.tile([128, 2 * TCW], F32, tag="scp")
                    nc.tensor.matmul(scp[:, 0:TCW], kpack[0:64, ks], qall[0:64, qs],
                                     start=True, stop=True, tile_position=(0, 0))
                    nc.tensor.matmul(scp[:, TCW:2 * TCW], kpack[64:128, ks], qall[64:128, qs],
                                     start=True, stop=True, tile_position=(64, 0))
                    ep = pw.tile([128, 2 * TCW], BF, tag="ep", bufs=5)
                    dj = kj - 4 * tci
                    for side in range(2):
                        sl = slice(side * TCW, (side + 1) * TCW)
                        nc.scalar.activation(ep[:, sl], scp[:, sl], AF.Exp, scale=SCALE)
                        if dj >= 0:
                            ms = slice(dj * 2 * TCW + side * TCW,
                                       dj * 2 * TCW + (side + 1) * TCW)
                            with tcx.high_priority():
                                nc.vector.tensor_mul(ep[:, sl], ep[:, sl], msk_t[:, ms])
                        col = (side * NKJ + kj) * 65
                        aug = augA if side == 0 else augB
                        nc.tensor.matmul(aug[:, :], vaug[:, col:col + 65], ep[:, sl],
                                         start=(kj == 0), stop=(kj == last_kj))
                    it += 1
                    e_end = max(1, (6 * total_iters) // 10)
                    want_e = len(early) * min(it, e_end) // e_end
                    while consumed_e < want_e:
                        early[consumed_e]()
                        consumed_e += 1
                    l_start = total_iters // 4
                    want_l = len(late) * max(0, it - l_start) // (total_iters - l_start)
                    while consumed_l < want_l:
                        late[consumed_l]()
                        consumed_l += 1
                # evacuate psum quickly so the next pack's accumulators start
                augSA = pw.tile([65, TCW], BF, tag="augSA", bufs=4)
                augSB = pw.tile([65, TCW], BF, tag="augSB", bufs=4)
                with tcx.high_priority():
                    nc.vector.tensor_copy(augSA[:, :], augA[:, :])
                    nc.vector.tensor_copy(augSB[:, :], augB[:, :])
                augs[p] = (augSA, augSB)
                # gather this pack's two softmax denominators right away
                if p == 0:
                    den8b = pw.tile([8, TCW], BF, tag="den8b", bufs=2)
                for i in (2 * p, 2 * p + 1):
                    nc.sync.dma_start(den8b[i:i + 1, :], augs[i // 2][i % 2][64:65, :])
            while consumed_e < len(early):
                early[consumed_e]()
                consumed_e += 1
            while consumed_l < len(late):
                late[consumed_l]()
                consumed_l += 1
            prev_pairs = norm_units_state["mk"](augs, den8b)

        for u in norm_units_state["pending"]:
            u()
        for lst in deferred_oproj:
            for u in lst:
                u()
        for u in oproj_units(TC - 1, prev_pairs):
            u()

    nc.compile()
    return nc


def _host_prep(x, Wqkv, Wo):
    """Build per-core input maps. Returns list of 8 dicts."""
    invfreq = 1.0 / (ROPE_BASE ** (np.arange(0, HD, 2, dtype=np.float32) / HD))
    ang = np.arange(L, dtype=np.float32)[:, None] * invfreq[None, :]   # [L, 32]
    cos = np.cos(ang).T     # [32, L]
    sin = np.sin(ang).T
    cc = np.tile(cos, (4, 1)).astype(np.float32)                       # [128, L]
    sgn = np.repeat(np.array([-1.0, 1.0, -1.0, 1.0], np.float32), 32)
    ss = (np.tile(sin, (4, 1)) * sgn[:, None]).astype(np.float32)
    ccss = np.concatenate([cc, ss], axis=1).astype(BF16)               # [128, 2L]

    r = np.arange(128)[:, None]
    c = np.arange(TCW)[None, :]
    msk = np.concatenate(
        [np.tile((r + 128 * j <= c).astype(np.float32), (1, 2)) for j in range(4)],
        axis=1).astype(BF16)                                           # [128, 4096]

    id2 = np.zeros((128, 64), np.float32)
    id2[:64] = np.eye(64, dtype=np.float32)
    id2[64:] = np.eye(64, dtype=np.float32)
    id2 = id2.astype(BF16)

    wq_part = Wqkv[:NH * HD].reshape(NH, HD, D)
    wk_part = Wqkv[NH * HD:NH * HD + NKV * HD].reshape(NKV, HD, D)
    wv_part = Wqkv[NH * HD + NKV * HD:].reshape(NKV, HD, D)

    in_maps = []
    for core in range(8):
        b, g = core // 4, core % 4
        xT = np.ascontiguousarray(x[b].T)                              # [D, L]
        xh = (xT.reshape(NKD, 128, TC, TCW).transpose(2, 1, 0, 3)
              .reshape(TC, 128, NKD * TCW)).astype(BF16)

        rows = []
        for p in range(4):
            rows.append(wq_part[8 * g + p])
            rows.append(wq_part[8 * g + 4 + p])
        rows.append(wk_part[2 * g]); rows.append(wk_part[2 * g + 1])
        rows.append(wv_part[2 * g]); rows.append(wv_part[2 * g + 1])
        Wc = np.concatenate(rows, axis=0)                              # [768, D]
        wq = (Wc.reshape(NOT, 128, NKD, 128).transpose(3, 0, 2, 1)
              .reshape(128, NOT * NKD * 128)).astype(BF16)

        cols = np.empty((4, 128), np.int64)
        for kt in range(4):
            cols[kt, :64] = (8 * g + kt) * HD + np.arange(64)
            cols[kt, 64:] = (8 * g + 4 + kt) * HD + np.arange(64)
        Woc = Wo.T[cols.reshape(-1)]                                   # [512, D]
        wo = (Woc.reshape(4, 128, NKD, 128).transpose(1, 0, 2, 3)
              .reshape(128, 4 * NKD * 128)).astype(BF16)

        in_maps.append(dict(xh=xh, wq=wq, wo=wo, ccss=ccss, msk=msk, id2=id2))
    return in_maps


def _get_nc():
    if "nc" not in _cached:
        _cached["nc"] = _build_nc()
    return _cached["nc"]


def run_sharded(x, Wqkv, Wo, trace=False):
    """Run on 8 cores; returns (out [B,L,D] float32, BassKernelResults)."""
    from concourse.bass_utils import run_bass_kernel_spmd
    nc = _get_nc()
    in_maps = _host_prep(np.asarray(x, np.float32), np.asarray(Wqkv, np.float32),
                         np.asarray(Wo, np.float32))
    res = run_bass_kernel_spmd(nc, in_maps, list(range(8)), trace=trace)
    out = np.zeros((B, L, D), np.float64)
    for core in range(8):
        b = core // 4
        P = res.results[core]["outp"].astype(np.float32).transpose(1, 2, 0, 3).reshape(D, L)
        out[b] += P.T.astype(np.float64)
    return out.astype(np.float32), res


def kernel(x, Wqkv, Wo):
    out, _ = run_sharded(x, Wqkv, Wo, trace=False)
    return out
